# revision 25
# baseline (speedup 1.0000x reference)
"""Trainium2 Bass kernel for nn_DecompMultiTransform (RGCN basis-decomposition).

Reference computation:
    full_w = (w_comp @ weight).reshape(64, 256, 256)   # per-type weights
    out[n, :] = x[n, :] @ full_w[xtype[n]]             # N = 4096

Scheme (type-parallel, minimal FLOPs):
  Host sorts rows by type into 64 zero-padded groups of CAP rows (pure
  layout - permutation, padding, transpose, bf16 cast). Core c owns types
  8c..8c+7. On device:

  Stage 1 - build this core's 8 per-type weight matrices on the PE:
      W_tau[i, o] = sum_b w_comp[tau, b] * weight[b, i*256+o]
    The contraction K packs (r=8 o-columns x b=16 bases) = 128 so the PE
    runs full-K matmuls:  lhsT = wstack_g[(r,b), j]  (a host re-layout of
    weight), rhs = cdelta[(r,b), (r', t)] which holds w_comp values
    delta-masked on r==r'. 64 matmuls of [K=128, M=128, N=64] produce
    W_tau[i, o] tiles with i on partitions; strided copies move banks
    PSUM->SBUF as bf16. Bank-pair 2q,2q+1 completes o-quarter q (both
    i-halves).

  Stage 2 - per type, per o-quarter: out_t[n, 64q:64q+64] accumulated
    over the two i-halves with x stationary (lhsT = xsT[i, n]). The
    o-quarter granularity lets stage-2 chase stage-1 chunk arrivals, and
    output halves drain (PSUM->SBUF copy -> DMA) as soon as both their
    quarters finish, so only the last o-quarter + one pair-drain chain
    trails the final weight chunk.

  DMA: inputs split over both HWDGE rings (sync: W quarters 0,2 + last
  bank; scalar: cdelta, x halves, W quarter 1 + bank 6) so both rings
  stream from the start of the program; outputs ride the same rings
  behind the inputs. All operands bf16 (PSUM accumulates f32). Host
  un-sorts the output.
"""

import sys

if "/opt/trn_rl_repo" not in sys.path:
    sys.path.insert(0, "/opt/trn_rl_repo")

import numpy as np

import concourse.bass as bass
import concourse.mybir as mybir
import concourse.tile as tile
from concourse import bacc
from concourse.bass_utils import run_bass_kernel_spmd

P = 128
N_FULL = 4096
IN_DIM = 256
OUT_DIM = 256
NUM_B = 16
NUM_T = 64
N_CORES = 8
TPC = NUM_T // N_CORES            # 8 types per core
NG = 64                           # stage-1 groups: (ih 2) x (og 32)
G_PER_BANK = 8                    # one PSUM bank = 8 groups

F32 = mybir.dt.float32
BF16 = mybir.dt.bfloat16
NP_BF16 = mybir.dt.np(BF16)




def _build_program(cap):
    nc = bacc.Bacc("TRN2", target_bir_lowering=False, debug=False)

    # xsT6[j, ih, t, n]: x value for type t, row-slot n, input
    # i = ih*128 + j. Contiguous per partition; one DMA.
    xsT = nc.declare_dram_parameter("xsT", [P, 2, TPC, cap], BF16,
                                    isOutput=False)
    # w0c: cdelta (64 cols) + weight bank 0, so the small cdelta rides
    # the front of a single fully-contiguous transfer. Remaining banks
    # ship one per DMA (b4/b5 fused) so the PE paces with arrivals.
    w0c = nc.declare_dram_parameter("w0c", [P, 8 * TPC + 8 * P], BF16,
                                    isOutput=False)
    wb1 = nc.declare_dram_parameter("wb1", [P, 8, P], BF16, isOutput=False)
    wb2 = nc.declare_dram_parameter("wb2", [P, 8, P], BF16, isOutput=False)
    wb3 = nc.declare_dram_parameter("wb3", [P, 8, P], BF16, isOutput=False)
    w45 = nc.declare_dram_parameter("w45", [P, 16, P], BF16, isOutput=False)
    wb6 = nc.declare_dram_parameter("wb6", [P, 8, P], BF16, isOutput=False)
    wb7 = nc.declare_dram_parameter("wb7", [P, 8, P], BF16, isOutput=False)
    # outb[h, gg, n, d, u, oc]: out for type t=4*gg+2*d+u, row n,
    # o = h*128+oc. One DMA per (h, gg) quarter of the output.
    outb = nc.declare_dram_parameter("outb", [2, 2, cap, 2, 2, P], BF16,
                                     isOutput=True)

    with tile.TileContext(nc) as tc:
        with (
            tc.tile_pool(name="const", bufs=1) as constp,
            tc.tile_pool(name="wpool", bufs=1) as wpool,
            tc.tile_pool(name="wsbp", bufs=1) as wsbp,
            tc.tile_pool(name="stp", bufs=4) as stp,
            tc.tile_pool(name="ps1", bufs=3, space="PSUM") as ps1,
            tc.tile_pool(name="pso", bufs=1, space="PSUM") as pso,
        ):
            w0t = constp.tile([P, 8 * TPC + 8 * P], BF16, name="w0t")
            xst = constp.tile([P, 2, TPC, cap], BF16, name="xst")
            wb1t = wpool.tile([P, 8, P], BF16, name="wb1t")
            wb2t = wpool.tile([P, 8, P], BF16, name="wb2t")
            wb3t = wpool.tile([P, 8, P], BF16, name="wb3t")
            w45t = wpool.tile([P, 16, P], BF16, name="w45t")
            wb6t = wpool.tile([P, 8, P], BF16, name="wb6t")
            wb7t = wpool.tile([P, 8, P], BF16, name="wb7t")
            cd = w0t[:, 0:8 * TPC]

            # ---- input DMA triggers. The scalar (ACT) HWDGE ring starts
            # ~1.8us late behind the activation-table load, so the weight
            # chunks that gate the PE ride the sync ring in program order;
            # the scalar ring carries x (not needed until stage 2) and the
            # mid-run quarter-2 banks.
            nc.sync.dma_start(out=w0t[:], in_=w0c.ap()[:, :])
            nc.scalar.dma_start(out=xst[:], in_=xsT.ap()[:, :, :, :])
            nc.sync.dma_start(out=wb1t[:], in_=wb1.ap()[:, :, :])
            nc.sync.dma_start(out=wb2t[:], in_=wb2.ap()[:, :, :])
            nc.scalar.dma_start(out=w45t[:], in_=w45.ap()[:, :, :])
            nc.sync.dma_start(out=wb3t[:], in_=wb3.ap()[:, :, :])
            nc.sync.dma_start(out=wb6t[:], in_=wb6.ap()[:, :, :])
            nc.sync.dma_start(out=wb7t[:], in_=wb7.ap()[:, :, :])

            def wslice(s):
                if s < 8:
                    return w0t[:, 8 * TPC + s * P: 8 * TPC + (s + 1) * P]
                for gs, sz, wt in ((8, 8, wb1t), (16, 8, wb2t), (24, 8, wb3t),
                                   (32, 16, w45t), (48, 8, wb6t),
                                   (56, 8, wb7t)):
                    if gs <= s < gs + sz:
                        return wt[:, s - gs, :]
                raise AssertionError(s)

            # quarter-major W store: wsb[ih][:, q, t, gl, rp] so the bank
            # copy writes one contiguous 512-col block and the stage-2 rhs
            # reads one contiguous 64-col block
            wsb = [
                wsbp.tile([P, 4, TPC, 8, 8], BF16, name=f"wsb{ih}")
                for ih in range(2)
            ]
            # one PSUM bank per type pair: po[pair][n, u, o]
            pos = [
                pso.tile([cap, 2, OUT_DIM], F32, name=f"po{i}", space="PSUM")
                for i in range(TPC // 2)
            ]

            def s1_bank(b):
                ps = ps1.tile([P, G_PER_BANK, 8 * TPC], F32, name="ps1",
                              tag="ps1", space="PSUM")
                for k in range(G_PER_BANK):
                    nc.tensor.matmul(out=ps[:, k, :], lhsT=wslice(b * 8 + k),
                                     rhs=cd, start=True, stop=True)
                # scatter bank into wsb[ih][:, q] (bf16); cd columns are
                # (t, rp)-ordered so both sides have contiguous 8-runs.
                # Split into gl-halves across DVE+ACT to halve the latency
                # between a bank's matmuls and its stage-2 wave.
                ih, q = b % 2, b // 2
                for gh, first in ((0, True), (1, False)):
                    src = ps[:, gh * 4:(gh + 1) * 4, :].rearrange(
                        "p gl (t rp) -> p t gl rp", rp=8, t=TPC)
                    dst = wsb[ih][:, q, :, gh * 4:(gh + 1) * 4, :]
                    if first:
                        nc.vector.tensor_copy(out=dst, in_=src)
                    else:
                        nc.scalar.copy(dst, src)

            def xs(ih, t):
                return xst[:, ih, t, :]

            def s2_mm(t, q, ih):
                nc.tensor.matmul(
                    out=pos[t // 2][:, t % 2, q * 64:(q + 1) * 64],
                    lhsT=xs(ih, t),
                    rhs=wsb[ih][:, q, t].rearrange("p gl rp -> p (gl rp)"),
                    start=(ih == 0),
                    stop=(ih == 1),
                )

            def drain_copy(st2, pr, h):
                # pair pr -> half d of the shared [cap, 2, 2, P] drain tile
                src = pos[pr][:, :, h * P:(h + 1) * P]
                if pr % 2 == 0:
                    nc.vector.tensor_copy(out=st2[:, pr % 2], in_=src)
                else:
                    nc.scalar.copy(st2[:, pr % 2], src)

            # ---- banks 0-5 with stage-2 quarters 0-2
            for b in range(6):
                s1_bank(b)
                if b % 2 == 1:
                    q = b // 2
                    for t in range(TPC):
                        s2_mm(t, q, 0)
                        s2_mm(t, q, 1)

            # ---- endgame: banks 6,7 first (q3 gates on their copies),
            # then the A-half drain copies slot in behind them on DVE/ACT
            # while the PE runs q3; B-half drains ride behind each pair.
            s1_bank(6)
            s1_bank(7)

            stA = [stp.tile([cap, 2, 2, P], BF16, name=f"stA{gg}", tag="st")
                   for gg in range(2)]
            for gg in range(2):
                drain_copy(stA[gg], 2 * gg, 0)
                drain_copy(stA[gg], 2 * gg + 1, 0)
            for gg in range(2):
                nc.gpsimd.dma_start(out=outb.ap()[0, gg], in_=stA[gg])

            stB = [stp.tile([cap, 2, 2, P], BF16, name=f"stB{gg}", tag="st")
                   for gg in range(2)]
            for pr in range(TPC // 2):
                for t in (2 * pr, 2 * pr + 1):
                    s2_mm(t, 3, 0)
                    s2_mm(t, 3, 1)
                gg, d = divmod(pr, 2)
                drain_copy(stB[gg], pr, 1)
                if d == 1:
                    # the two B-half output DMAs go to different rings so
                    # the final triggers issue in parallel
                    deng = nc.sync if gg == 0 else nc.scalar
                    deng.dma_start(out=outb.ap()[1, gg], in_=stB[gg])

    nc.compile()
    return nc


_PROGRAMS = {}
LAST_RESULT = None  # test harness introspection


def kernel(x, xtype, weight, w_comp, trace=False):
    global LAST_RESULT
    x = np.asarray(x, dtype=np.float32)
    xtype = np.asarray(xtype).astype(np.int64)
    weight = np.asarray(weight, dtype=np.float32)
    w_comp = np.asarray(w_comp, dtype=np.float32)
    assert x.shape == (N_FULL, IN_DIM) and weight.shape == (NUM_B, IN_DIM * OUT_DIM)

    # ---- host-side layout: sort rows by type into padded slots ----
    counts = np.bincount(xtype, minlength=NUM_T)
    cap = int(-(-max(counts.max(), 32) // 8) * 8)
    if cap > P:
        raise RuntimeError(f"type count {counts.max()} exceeds {P}")
    if cap not in _PROGRAMS:
        _PROGRAMS[cap] = _build_program(cap)
    nc = _PROGRAMS[cap]

    order = np.argsort(xtype, kind="stable")
    sorted_t = xtype[order]
    starts = np.zeros(NUM_T, dtype=np.int64)
    starts[1:] = np.cumsum(counts)[:-1]
    rank = np.arange(N_FULL, dtype=np.int64) - starts[sorted_t]
    slot = sorted_t * cap + rank  # global padded slot per sorted row

    xpad = np.zeros((NUM_T * cap, IN_DIM), np.float32)
    xpad[slot] = x[order]
    xpad = xpad.astype(NP_BF16)

    # wstack[(r,b), g, j] = weight[b, (ih*128+j)*256 + og*8+r], with the
    # g axis permuted into bank order: bank b = (o-quarter b//2, ih b%2),
    # slot k = og within quarter.
    w5 = weight.reshape(NUM_B, 2, P, 32, 8)  # b, ih, j, og, r
    wst_nat = np.ascontiguousarray(w5.transpose(4, 0, 1, 3, 2)).reshape(P, NG, P)
    perm = np.empty(NG, np.int64)
    for s in range(NG):
        b, k = divmod(s, G_PER_BANK)
        ih, og = b % 2, (b // 2) * 8 + k
        perm[s] = ih * 32 + og
    wstack = np.ascontiguousarray(wst_nat[:, perm, :]).astype(NP_BF16)

    c_bf = w_comp.astype(NP_BF16)

    wb1c = np.ascontiguousarray(wstack[:, 8:16, :])
    wb2c = np.ascontiguousarray(wstack[:, 16:24, :])
    wb3c = np.ascontiguousarray(wstack[:, 24:32, :])
    w45c = np.ascontiguousarray(wstack[:, 32:48, :])
    wb6c = np.ascontiguousarray(wstack[:, 48:56, :])
    wb7c = np.ascontiguousarray(wstack[:, 56:64, :])

    in_maps = []
    for c in range(N_CORES):
        xc = xpad[c * TPC * cap:(c + 1) * TPC * cap]  # [8*cap, 256] bf16
        # [j, ih, t, n]
        xsT = np.ascontiguousarray(
            xc.reshape(TPC, cap, 2, P).transpose(3, 2, 0, 1)
        )
        cdl = np.zeros((8, NUM_B, TPC, 8), NP_BF16)  # r, b, t, rp
        for r in range(8):
            cdl[r, :, :, r] = c_bf[c * TPC:(c + 1) * TPC, :].T
        w0cc = np.concatenate(
            [cdl.reshape(P, 8 * TPC), wstack[:, 0:8, :].reshape(P, 8 * P)],
            axis=1,
        )
        in_maps.append(
            {
                "xsT": xsT,
                "w0c": np.ascontiguousarray(w0cc),
                "wb1": wb1c,
                "wb2": wb2c,
                "wb3": wb3c,
                "w45": w45c,
                "wb6": wb6c,
                "wb7": wb7c,
            }
        )

    res = run_bass_kernel_spmd(nc, in_maps, list(range(N_CORES)), trace=trace)
    LAST_RESULT = res

    out = np.empty((N_FULL, OUT_DIM), np.float32)
    for c in range(N_CORES):
        # [h, gg, n, d, u, oc] -> [gg, d, u, n, h, oc] -> [8*cap, 256]
        big = res.results[c]["outb"].transpose(1, 3, 4, 2, 0, 5).reshape(
            TPC * cap, OUT_DIM)
        sel = (slot >= c * TPC * cap) & (slot < (c + 1) * TPC * cap)
        out[order[sel]] = big[slot[sel] - c * TPC * cap].astype(np.float32)
    return out


# revision 30
# speedup vs baseline: 1.2192x; 1.2192x over previous
"""Trainium2 Bass kernel for nn_DecompMultiTransform (RGCN basis-decomposition).

Reference computation:
    full_w = (w_comp @ weight).reshape(64, 256, 256)   # per-type weights
    out[n, :] = x[n, :] @ full_w[xtype[n]]             # N = 4096

Scheme (type-parallel, minimal FLOPs):
  Host sorts rows by type into 64 zero-padded groups of CAP rows (pure
  layout - permutation, padding, transpose, bf16 cast). Core c owns types
  8c..8c+7. On device:

  Stage 1 - build this core's 8 per-type weight matrices on the PE:
      W_tau[i, o] = sum_b w_comp[tau, b] * weight[b, i*256+o]
    The contraction K packs (r=8 o-columns x b=16 bases) = 128 so the PE
    runs full-K matmuls:  lhsT = wstack_g[(r,b), j]  (a host re-layout of
    weight), rhs = cdelta[(r,b), (r', t)] which holds w_comp values
    delta-masked on r==r'. 64 matmuls of [K=128, M=128, N=64] produce
    W_tau[i, o] tiles with i on partitions; strided copies move banks
    PSUM->SBUF as bf16. Bank-pair 2q,2q+1 completes o-quarter q (both
    i-halves).

  Stage 2 - per type, per o-quarter: out_t[n, 64q:64q+64] accumulated
    over the two i-halves with x stationary (lhsT = xsT[i, n]). The
    o-quarter granularity lets stage-2 chase stage-1 chunk arrivals, and
    output halves drain (PSUM->SBUF copy -> DMA) as soon as both their
    quarters finish, so only the last o-quarter + one pair-drain chain
    trails the final weight chunk.

  DMA: inputs split over both HWDGE rings (sync: W quarters 0,2 + last
  bank; scalar: cdelta, x halves, W quarter 1 + bank 6) so both rings
  stream from the start of the program; outputs ride the same rings
  behind the inputs. All operands bf16 (PSUM accumulates f32). Host
  un-sorts the output.
"""

import sys

if "/opt/trn_rl_repo" not in sys.path:
    sys.path.insert(0, "/opt/trn_rl_repo")

import numpy as np

import concourse.bass as bass
import concourse.mybir as mybir
import concourse.tile as tile
from concourse import bacc
from concourse.bass_utils import run_bass_kernel_spmd

P = 128
N_FULL = 4096
IN_DIM = 256
OUT_DIM = 256
NUM_B = 16
NUM_T = 64
N_CORES = 8
TPC = NUM_T // N_CORES            # 8 types per core
NG = 64                           # stage-1 groups: (ih 2) x (og 32)
G_PER_BANK = 8                    # one PSUM bank = 8 groups

F32 = mybir.dt.float32
BF16 = mybir.dt.bfloat16
NP_BF16 = mybir.dt.np(BF16)




def _build_program(cap):
    nc = bacc.Bacc("TRN2", target_bir_lowering=False, debug=False)

    # xsT6[j, ih, t, n]: x value for type t, row-slot n, input
    # i = ih*128 + j. Contiguous per partition; one DMA.
    xsT = nc.declare_dram_parameter("xsT", [P, 2, TPC, cap], BF16,
                                    isOutput=False)
    # w0c: cdelta (64 cols) + weight banks 0-1, so the small cdelta
    # rides the front of a single large fully-contiguous transfer.
    w0c = nc.declare_dram_parameter("w0c", [P, 8 * TPC + 16 * P], BF16,
                                    isOutput=False)
    w1 = nc.declare_dram_parameter("w1", [P, 16, P], BF16, isOutput=False)
    w2 = nc.declare_dram_parameter("w2", [P, 16, P], BF16, isOutput=False)
    w3a = nc.declare_dram_parameter("w3a", [P, 8, P], BF16, isOutput=False)
    w3b = nc.declare_dram_parameter("w3b", [P, 8, P], BF16, isOutput=False)
    # outb[h, gg, n, d, u, oc]: out for type t=4*gg+2*d+u, row n,
    # o = h*128+oc. One DMA per (h, gg) quarter of the output.
    outb = nc.declare_dram_parameter("outb", [2, 2, cap, 2, 2, P], BF16,
                                     isOutput=True)

    with tile.TileContext(nc) as tc:
        with (
            tc.tile_pool(name="const", bufs=1) as constp,
            tc.tile_pool(name="wpool", bufs=1) as wpool,
            tc.tile_pool(name="wsbp", bufs=1) as wsbp,
            tc.tile_pool(name="stp", bufs=4) as stp,
            tc.tile_pool(name="ps1", bufs=3, space="PSUM") as ps1,
            tc.tile_pool(name="pso", bufs=1, space="PSUM") as pso,
        ):
            w0t = constp.tile([P, 8 * TPC + 16 * P], BF16, name="w0t")
            xst = constp.tile([P, 2, TPC, cap], BF16, name="xst")
            w1t = wpool.tile([P, 16, P], BF16, name="w1t")
            w2t = wpool.tile([P, 16, P], BF16, name="w2t")
            w3at = wpool.tile([P, 8, P], BF16, name="w3at")
            w3bt = wpool.tile([P, 8, P], BF16, name="w3bt")
            cd = w0t[:, 0:8 * TPC]

            # ---- input DMA triggers. The scalar (ACT) HWDGE ring starts
            # ~1.8us late behind the activation-table load, so the weight
            # chunks that gate the PE ride the sync ring in program order;
            # the scalar ring carries x (not needed until stage 2) and the
            # mid-run quarter-2 banks.
            nc.sync.dma_start(out=w0t[:], in_=w0c.ap()[:, :])
            nc.scalar.dma_start(out=xst[:], in_=xsT.ap()[:, :, :, :])
            nc.sync.dma_start(out=w1t[:], in_=w1.ap()[:, :, :])
            nc.scalar.dma_start(out=w2t[:], in_=w2.ap()[:, :, :])
            nc.sync.dma_start(out=w3at[:], in_=w3a.ap()[:, :, :])
            nc.sync.dma_start(out=w3bt[:], in_=w3b.ap()[:, :, :])

            def wslice(s):
                if s < 16:
                    return w0t[:, 8 * TPC + s * P: 8 * TPC + (s + 1) * P]
                for gs, sz, wt in ((16, 16, w1t), (32, 16, w2t),
                                   (48, 8, w3at), (56, 8, w3bt)):
                    if gs <= s < gs + sz:
                        return wt[:, s - gs, :]
                raise AssertionError(s)

            # quarter-major W store: wsb[ih][:, q, t, gl, rp] so the bank
            # copy writes one contiguous 512-col block and the stage-2 rhs
            # reads one contiguous 64-col block
            wsb = [
                wsbp.tile([P, 4, TPC, 8, 8], BF16, name=f"wsb{ih}")
                for ih in range(2)
            ]
            # one PSUM bank per type pair: po[pair][n, u, o]
            pos = [
                pso.tile([cap, 2, OUT_DIM], F32, name=f"po{i}", space="PSUM")
                for i in range(TPC // 2)
            ]

            def s1_bank(b):
                ps = ps1.tile([P, G_PER_BANK, 8 * TPC], F32, name="ps1",
                              tag="ps1", space="PSUM")
                for k in range(G_PER_BANK):
                    nc.tensor.matmul(out=ps[:, k, :], lhsT=wslice(b * 8 + k),
                                     rhs=cd, start=True, stop=True)
                # scatter bank into wsb[ih][:, q] (bf16); cd columns are
                # (t, rp)-ordered so both sides have contiguous 8-runs
                ih, q = b % 2, b // 2
                src = ps[:].rearrange("p gl (t rp) -> p t gl rp", rp=8, t=TPC)
                dst = wsb[ih][:, q]
                if b % 2 == 0:
                    nc.vector.tensor_copy(out=dst, in_=src)
                else:
                    nc.scalar.copy(dst, src)

            def xs(ih, t):
                return xst[:, ih, t, :]

            def s2_mm(t, q, ih):
                nc.tensor.matmul(
                    out=pos[t // 2][:, t % 2, q * 64:(q + 1) * 64],
                    lhsT=xs(ih, t),
                    rhs=wsb[ih][:, q, t].rearrange("p gl rp -> p (gl rp)"),
                    start=(ih == 0),
                    stop=(ih == 1),
                )

            def drain_copy(st2, pr, h):
                # pair pr -> half d of the shared [cap, 2, 2, P] drain tile
                src = pos[pr][:, :, h * P:(h + 1) * P]
                if pr % 2 == 0:
                    nc.vector.tensor_copy(out=st2[:, pr % 2], in_=src)
                else:
                    nc.scalar.copy(st2[:, pr % 2], src)

            # ---- banks 0-5 with stage-2 quarters 0-2
            for b in range(6):
                s1_bank(b)
                if b % 2 == 1:
                    q = b // 2
                    for t in range(TPC):
                        s2_mm(t, q, 0)
                        s2_mm(t, q, 1)

            # ---- endgame: banks 6,7 first (q3 gates on their copies),
            # then the A-half drain copies slot in behind them on DVE/ACT
            # while the PE runs q3; B-half drains ride behind each pair.
            s1_bank(6)
            s1_bank(7)

            stA = [stp.tile([cap, 2, 2, P], BF16, name=f"stA{gg}", tag="st")
                   for gg in range(2)]
            for gg in range(2):
                drain_copy(stA[gg], 2 * gg, 0)
                drain_copy(stA[gg], 2 * gg + 1, 0)
            for gg in range(2):
                nc.gpsimd.dma_start(out=outb.ap()[0, gg], in_=stA[gg])

            stB = [stp.tile([cap, 2, 2, P], BF16, name=f"stB{gg}", tag="st")
                   for gg in range(2)]
            for pr in range(TPC // 2):
                for t in (2 * pr, 2 * pr + 1):
                    s2_mm(t, 3, 0)
                    s2_mm(t, 3, 1)
                gg, d = divmod(pr, 2)
                drain_copy(stB[gg], pr, 1)
                if d == 1:
                    # the two B-half output DMAs go to different rings so
                    # the final triggers issue in parallel
                    deng = nc.sync if gg == 0 else nc.scalar
                    deng.dma_start(out=outb.ap()[1, gg], in_=stB[gg])

    nc.compile()
    return nc


_PROGRAMS = {}
LAST_RESULT = None  # test harness introspection


def kernel(x, xtype, weight, w_comp, trace=False):
    global LAST_RESULT
    x = np.asarray(x, dtype=np.float32)
    xtype = np.asarray(xtype).astype(np.int64)
    weight = np.asarray(weight, dtype=np.float32)
    w_comp = np.asarray(w_comp, dtype=np.float32)
    assert x.shape == (N_FULL, IN_DIM) and weight.shape == (NUM_B, IN_DIM * OUT_DIM)

    # ---- host-side layout: sort rows by type into padded slots ----
    counts = np.bincount(xtype, minlength=NUM_T)
    cap = int(-(-max(counts.max(), 32) // 8) * 8)
    if cap > P:
        raise RuntimeError(f"type count {counts.max()} exceeds {P}")
    if cap not in _PROGRAMS:
        _PROGRAMS[cap] = _build_program(cap)
    nc = _PROGRAMS[cap]

    order = np.argsort(xtype, kind="stable")
    sorted_t = xtype[order]
    starts = np.zeros(NUM_T, dtype=np.int64)
    starts[1:] = np.cumsum(counts)[:-1]
    rank = np.arange(N_FULL, dtype=np.int64) - starts[sorted_t]
    slot = sorted_t * cap + rank  # global padded slot per sorted row

    xpad = np.zeros((NUM_T * cap, IN_DIM), np.float32)
    xpad[slot] = x[order]
    xpad = xpad.astype(NP_BF16)

    # wstack[(r,b), g, j] = weight[b, (ih*128+j)*256 + og*8+r], with the
    # g axis permuted into bank order: bank b = (o-quarter b//2, ih b%2),
    # slot k = og within quarter.
    w5 = weight.reshape(NUM_B, 2, P, 32, 8)  # b, ih, j, og, r
    wst_nat = np.ascontiguousarray(w5.transpose(4, 0, 1, 3, 2)).reshape(P, NG, P)
    perm = np.empty(NG, np.int64)
    for s in range(NG):
        b, k = divmod(s, G_PER_BANK)
        ih, og = b % 2, (b // 2) * 8 + k
        perm[s] = ih * 32 + og
    wstack = np.ascontiguousarray(wst_nat[:, perm, :]).astype(NP_BF16)

    c_bf = w_comp.astype(NP_BF16)

    w1c = np.ascontiguousarray(wstack[:, 16:32, :])
    w2c = np.ascontiguousarray(wstack[:, 32:48, :])
    w3ac = np.ascontiguousarray(wstack[:, 48:56, :])
    w3bc = np.ascontiguousarray(wstack[:, 56:64, :])

    in_maps = []
    for c in range(N_CORES):
        xc = xpad[c * TPC * cap:(c + 1) * TPC * cap]  # [8*cap, 256] bf16
        # [j, ih, t, n]
        xsT = np.ascontiguousarray(
            xc.reshape(TPC, cap, 2, P).transpose(3, 2, 0, 1)
        )
        cdl = np.zeros((8, NUM_B, TPC, 8), NP_BF16)  # r, b, t, rp
        for r in range(8):
            cdl[r, :, :, r] = c_bf[c * TPC:(c + 1) * TPC, :].T
        w0cc = np.concatenate(
            [cdl.reshape(P, 8 * TPC), wstack[:, 0:16, :].reshape(P, 16 * P)],
            axis=1,
        )
        in_maps.append(
            {
                "xsT": xsT,
                "w0c": np.ascontiguousarray(w0cc),
                "w1": w1c,
                "w2": w2c,
                "w3a": w3ac,
                "w3b": w3bc,
            }
        )

    res = run_bass_kernel_spmd(nc, in_maps, list(range(N_CORES)), trace=trace)
    LAST_RESULT = res

    out = np.empty((N_FULL, OUT_DIM), np.float32)
    for c in range(N_CORES):
        # [h, gg, n, d, u, oc] -> [gg, d, u, n, h, oc] -> [8*cap, 256]
        big = res.results[c]["outb"].transpose(1, 3, 4, 2, 0, 5).reshape(
            TPC * cap, OUT_DIM)
        sel = (slot >= c * TPC * cap) & (slot < (c + 1) * TPC * cap)
        out[order[sel]] = big[slot[sel] - c * TPC * cap].astype(np.float32)
    return out
